# revision 38
# baseline (speedup 1.0000x reference)
"""GCN layer (x@W, sparse-adj aggregate, +bias) on 8 Trainium2 NeuronCores.

Strategy (memory-regime), v4:
  - out = A @ (x @ W) + b = (A @ x) @ W + b: aggregate raw x rows first,
    project once per 128-row destination window.  No projected table, no
    phase A.
  - Destination nodes sharded 12500/core (1D partition per hint); edges
    sharded by destination row and sorted by (window-group, source-chunk,
    window), padded to 128-edge tiles per (group, chunk) segment.
  - Edge x-rows (256B bf16) are fetched with BULK dma_gather: 1024 rows
    per SWDGE instruction, round-robined over 4 SWDGE queues (descriptor
    generation parallelizes ~2.7x across queue contexts; ~2.4ns/row vs
    the baseline's 994ns-fixed-per-128-row indirect DMA).  int16 gather
    indices only address 32768 rows, hence the 4 source chunks.
  - Scatter into destination rows via one-hot matmuls: S[e, r] = val_e *
    (rloc_e == r) tiles are PREBUILT ON THE HOST and streamed in via
    HWDGE (keeps DVE/GpSimd off the shared SBUF port pair that SWDGE
    descriptor generation needs; on-chip tensor_scalar builds serialized
    against the gathers).  PE matmul aggT[k, r] += X_tile[e, k]^T S[e, r]
    accumulates in PSUM per window; tiles straddling a window boundary
    issue one matmul per overlapped window (out-of-window edges are zero
    columns in S), which also makes the traced program core-uniform
    (window spans unioned across cores).
  - Projection = W-stationary matmul per 4 windows; bias is fused into
    the PSUM-evacuating activation on the Scalar engine.  Host undoes
    the layout permutation.
"""

import math
import os
import sys

import numpy as np

for _p in ("/opt/trn_rl_repo",):
    if _p not in sys.path:
        sys.path.insert(0, _p)

import ml_dtypes  # noqa: E402

from concourse import bacc, bass, library_config, mybir, tile  # noqa: E402
from concourse import bass_utils  # noqa: E402

BF16 = mybir.dt.bfloat16
F32 = mybir.dt.float32
I16 = mybir.dt.int16
NP_BF16 = ml_dtypes.bfloat16

P = 128
CH = 32768  # rows addressable by one int16-indexed gather


def default_cfg():
    return dict(
        n_nodes=100000,
        n_edges=800000,
        in_f=128,
        out_f=64,
        n_cores=8,
        gw=8,  # windows per PSUM group
        n_swdge_queues=4,  # parallel SWDGE descriptor-generation contexts
        gather_tiles=7,  # tiles per dma_gather call (896 idx)
        gather_sp=True,  # packetized descriptors (<=64/packet at 7 tiles)
        gbufs=2,  # rotating gather/S buffers (first gbufs groups gather pads)
    )


def _derived(cfg):
    n_nodes = cfg["n_nodes"]
    c = cfg["n_cores"]
    ns = n_nodes // c  # dest rows per core
    nw = math.ceil(ns / P)  # dest windows per core
    ng = math.ceil(nw / cfg["gw"])  # window groups
    nch = math.ceil(n_nodes / CH)  # source chunks
    return ns, nw, ng, nch


def prep_inputs(x, weights, bias, adj_rows, adj_cols, adj_vals, cfg):
    """Host-side sharding/index prep (numpy only). Returns (in_maps, meta)."""
    c = cfg["n_cores"]
    out_f = cfg["out_f"]
    in_f = cfg["in_f"]
    gw = cfg["gw"]
    n_nodes = cfg["n_nodes"]
    ns, nw, ng, nch = _derived(cfg)

    x = np.asarray(x, dtype=np.float32)
    weights = np.asarray(weights, dtype=np.float32)
    bias = np.asarray(bias, dtype=np.float32)
    rows = np.asarray(adj_rows).astype(np.int64)
    cols = np.asarray(adj_cols).astype(np.int64)
    vals = np.asarray(adj_vals, dtype=np.float32)

    xbf = np.ascontiguousarray(x.astype(NP_BF16))
    wt = weights.astype(NP_BF16)
    biascol = np.ascontiguousarray(bias[:, None]).astype(np.float32)  # [out_f, 1]
    iota = np.ascontiguousarray(
        np.broadcast_to(np.arange(P, dtype=np.float32), (P, P)).astype(NP_BF16)
    )

    # shard edges by destination core (contiguous row ranges)
    order = np.argsort(rows, kind="stable")
    rows_s, cols_s, vals_s = rows[order], cols[order], vals[order]
    core_start = np.searchsorted(rows_s, np.arange(c + 1) * ns)

    # per-core edge orderings and per-(group,chunk) segment sizes
    per_core = []
    cnt = np.zeros((c, ng, nch), dtype=np.int64)
    for ci in range(c):
        s, e = core_start[ci], core_start[ci + 1]
        rloc = rows_s[s:e] - ci * ns
        w = rloc // P
        g = w // gw
        ch = cols_s[s:e] // CH
        o2 = np.lexsort((w, ch, g))
        rloc, w, g, ch = rloc[o2], w[o2], g[o2], ch[o2]
        cloc = (cols_s[s:e] - (cols_s[s:e] // CH) * CH)[o2]
        vv = vals_s[s:e][o2]
        np.add.at(cnt[ci], (g, ch), 1)
        per_core.append((rloc, w, g, ch, cloc, vv))

    # core-uniform tile counts per (group, chunk)
    tiles_gc = np.maximum.reduce([-(-cnt[ci] // P) for ci in range(c)])  # [ng, nch]
    seg_off = np.zeros((ng, nch), dtype=np.int64)  # tile offset of segment in group
    idx_off = np.zeros((ng, nch), dtype=np.int64)  # idx col offset of segment (global)
    toff_g = np.zeros(ng + 1, dtype=np.int64)  # global tile offset of group
    icol = 0
    for gi in range(ng):
        t = 0
        for chi in range(nch):
            seg_off[gi, chi] = t
            idx_off[gi, chi] = icol
            t += tiles_gc[gi, chi]
            icol += tiles_gc[gi, chi] * (P // 16)
        toff_g[gi + 1] = toff_g[gi] + t
    ntiles = int(toff_g[-1])
    idxcols = ntiles * (P // 16)

    # per-core padded per-tile arrays. Pad gather slots are -1: the SWDGE
    # gather skips trailing negative indices (no descriptor, no bytes);
    # their S columns are zero so the stale SBUF data multiplies away.
    PAD_RLOC = -1.0e4
    cloc_pad = np.full((c, ntiles * P), -1, dtype=np.int16)
    rloc_pad = np.full((c, ntiles * P), PAD_RLOC, dtype=np.float32)
    vals_pad = np.zeros((c, ntiles * P), dtype=np.float32)
    wmin = np.full((c, ntiles), 1 << 30, dtype=np.int64)
    wmax = np.full((c, ntiles), -1, dtype=np.int64)
    for ci in range(c):
        rloc, w, g, ch, cloc, vv = per_core[ci]
        seg_start = np.searchsorted(g * nch + ch, np.arange(ng * nch))
        seg_start = np.append(seg_start, len(g))
        for gi in range(ng):
            for chi in range(nch):
                s0, e0 = seg_start[gi * nch + chi], seg_start[gi * nch + chi + 1]
                n = e0 - s0
                if n == 0:
                    continue
                t0 = (toff_g[gi] + seg_off[gi, chi]) * P
                cloc_pad[ci, t0 : t0 + n] = cloc[s0:e0].astype(np.int16)
                rloc_pad[ci, t0 : t0 + n] = rloc[s0:e0].astype(np.float32)
                vals_pad[ci, t0 : t0 + n] = vv[s0:e0]
                tt = toff_g[gi] + seg_off[gi, chi] + np.arange(n) // P
                np.minimum.at(wmin[ci], tt, w[s0:e0])
                np.maximum.at(wmax[ci], tt, w[s0:e0])

    # union window span per tile across cores -> op list (core-uniform)
    lo = wmin.min(axis=0)
    hi = wmax.max(axis=0)
    assert (hi >= 0).all() and (lo < (1 << 30)).all(), "tile with no real edges"

    # ops per group, sorted by (window, tile); start/stop per window
    ops_by_group = []  # list of list[(tile, w, start, stop)]
    nops = 0
    for gi in range(ng):
        t0, t1 = toff_g[gi], toff_g[gi + 1]
        ops = []
        for t in range(t0, t1):
            for w in range(lo[t], hi[t] + 1):
                ops.append((t, w))
        ops.sort(key=lambda tw: (tw[1], tw[0]))
        w_present = {w for _, w in ops}
        for w in range(gi * gw, min((gi + 1) * gw, nw)):
            assert w in w_present, f"window {w} has no ops"
        out = []
        for i, (t, w) in enumerate(ops):
            start = i == 0 or ops[i - 1][1] != w
            stop = i == len(ops) - 1 or ops[i + 1][1] != w
            out.append((t, w, start, stop))
        ops_by_group.append(out)
        nops += len(out)

    # per-core host-prebuilt one-hot S tiles (S[e, j] = val_e at j = rloc_e - 128w)
    in_maps = []
    for ci in range(c):
        smat = np.zeros((nops, P, P), dtype=NP_BF16)  # [op, e, r]
        o = 0
        for gi in range(ng):
            for t, w, _, _ in ops_by_group[gi]:
                rlo = rloc_pad[ci, t * P : (t + 1) * P] - np.float32(P * w)
                vvo = vals_pad[ci, t * P : (t + 1) * P]
                m = (rlo >= 0) & (rlo < P)
                e = np.nonzero(m)[0]
                smat[o, e, rlo[m].astype(np.int64)] = vvo[m].astype(NP_BF16)
                o += 1
        # DRAM layout [P(e), nops*P(r)]
        s_all = np.ascontiguousarray(smat.transpose(1, 0, 2).reshape(P, nops * P))
        idx16 = np.zeros((P, idxcols), dtype=np.int16)
        for gi in range(ng):
            for chi in range(nch):
                nt = tiles_gc[gi, chi]
                if nt == 0:
                    continue
                t0 = (toff_g[gi] + seg_off[gi, chi]) * P
                seg = cloc_pad[ci, t0 : t0 + nt * P]
                blk = np.tile(seg.reshape(-1, 16).T, (8, 1))  # [128, nt*8]
                ic = idx_off[gi, chi]
                idx16[:, ic : ic + nt * (P // 16)] = blk
        in_maps.append(
            dict(
                xbf=xbf,
                wt=wt,
                biascol=biascol,
                gidx=idx16,
                smat=s_all,
            )
        )

    # per-gather-call valid-index counts (trailing -1 pads are skipped by
    # the HW; count must be >=1, so all-pad calls keep one idx-0 slot).
    # Call structure must mirror build()'s issue_gathers loop exactly.
    cap = cfg.get("gather_tiles", 8)
    calls = []
    for gi in range(ng):
        for chi in range(nch):
            nt = int(tiles_gc[gi, chi])
            if nt == 0:
                continue
            for j in range(0, nt, cap):
                calls.append((gi, chi, j, min(cap, nt - j)))
    ncalls = len(calls)
    gcnt = np.zeros((c, ncalls), dtype=np.int32)
    full_call = [False] * ncalls
    for k, (gi, chi, j, nj) in enumerate(calls):
        t0 = toff_g[gi] + seg_off[gi, chi]
        if gi < cfg.get("gbufs", 3):
            # first use of each rotating gather buffer: gather the pads
            # (idx 0) instead of skipping, so the buffer never holds
            # non-finite garbage; later groups overwrite with real rows
            # or inherit finite stale values
            for ci in range(c):
                s = (t0 + j) * P
                seg = cloc_pad[ci, s : s + nj * P]
                seg[seg < 0] = 0
                gcnt[ci, k] = nj * P
            full_call[k] = True
            continue
        for ci in range(c):
            real = int(cnt[ci, gi, chi])
            valid = min(max(real - j * P, 0), nj * P)
            if valid == 0:
                cloc_pad[ci, (t0 + j) * P] = 0
                valid = 1
            gcnt[ci, k] = valid
        full_call[k] = all(gcnt[ci, k] == nj * P for ci in range(c))
    for ci in range(c):
        in_maps[ci]["gcnt"] = np.ascontiguousarray(gcnt[ci : ci + 1])
        # rebuild idx16 (cloc_pad may have gained idx-0 slots for empty calls)
        idx16 = in_maps[ci]["gidx"]
        for gi in range(ng):
            for chi in range(nch):
                nt = tiles_gc[gi, chi]
                if nt == 0:
                    continue
                t0 = (toff_g[gi] + seg_off[gi, chi]) * P
                seg = cloc_pad[ci, t0 : t0 + nt * P]
                blk = np.tile(seg.reshape(-1, 16).T, (8, 1))
                ic = idx_off[gi, chi]
                idx16[:, ic : ic + nt * (P // 16)] = blk

    meta = dict(
        tiles_gc=tiles_gc.tolist(),
        seg_off=seg_off.tolist(),
        idx_off=idx_off.tolist(),
        toff_g=toff_g.tolist(),
        ntiles=ntiles,
        idxcols=idxcols,
        nops=nops,
        ops_by_group=ops_by_group,
        ncalls=ncalls,
        full_call=full_call,
    )
    return in_maps, meta


def build(nc, meta, cfg):
    """Trace the (per-core identical) kernel program."""
    out_f = cfg["out_f"]
    in_f = cfg["in_f"]
    gw = cfg["gw"]
    n_nodes = cfg["n_nodes"]
    pattern = cfg.get("sb_pattern", "v")
    ns, nw, ng, nch = _derived(cfg)
    assert in_f == P
    tiles_gc = meta["tiles_gc"]
    seg_off = meta["seg_off"]
    idx_off = meta["idx_off"]
    toff_g = meta["toff_g"]
    ntiles = meta["ntiles"]
    idxcols = meta["idxcols"]
    nops = meta["nops"]
    ops_by_group = meta["ops_by_group"]
    tmax = max(toff_g[gi + 1] - toff_g[gi] for gi in range(ng))
    nproj = math.ceil(gw / 4)  # projection matmuls per group (N=512)

    xbf_d = nc.dram_tensor("xbf", [n_nodes, in_f], BF16, kind="ExternalInput")
    wt_d = nc.dram_tensor("wt", [P, out_f], BF16, kind="ExternalInput")
    bias_d = nc.dram_tensor("biascol", [out_f, 1], F32, kind="ExternalInput")
    gidx_d = nc.dram_tensor("gidx", [P, idxcols], I16, kind="ExternalInput")
    smat_d = nc.dram_tensor("smat", [P, nops * P], BF16, kind="ExternalInput")
    out_d = nc.dram_tensor("out", [out_f, nw * P], F32, kind="ExternalOutput")
    ops_per_g = [len(ops_by_group[gi]) for gi in range(ng)]
    obase_g = [sum(ops_per_g[:gi]) for gi in range(ng)]
    omax = max(ops_per_g)
    ncalls = meta["ncalls"]
    full_call = meta["full_call"]
    gcnt_d = nc.dram_tensor("gcnt", [1, ncalls], mybir.dt.int32, kind="ExternalInput")
    stage = cfg.get("stage", 99)
    if stage <= 1:
        gdump_d = nc.dram_tensor("gdump", [P, ntiles * P], BF16, kind="ExternalOutput")

    copyf = mybir.ActivationFunctionType.Copy

    with tile.TileContext(nc) as tc:
        with (
            tc.tile_pool(name="const", bufs=1) as cpool,
            tc.tile_pool(name="gbuf", bufs=cfg.get("gbufs", 3)) as gpool,
            tc.tile_pool(name="sdma", bufs=2) as spool,
            tc.tile_pool(name="apsum", bufs=2, space="PSUM") as apool,
            tc.tile_pool(name="agsb", bufs=2) as agpool,
            tc.tile_pool(name="ppsum", bufs=2, space="PSUM") as ppool,
            tc.tile_pool(name="osb", bufs=2) as opool,
        ):
            nc.gpsimd.load_library(library_config.mlp)
            wt_t = cpool.tile([P, out_f], BF16)
            nc.sync.dma_start(out=wt_t[:], in_=wt_d[:])
            bias_t = cpool.tile([out_f, 1], F32)
            nc.sync.dma_start(out=bias_t[:], in_=bias_d[:])
            idx_t = cpool.tile([P, idxcols], I16)
            gcnt_t = cpool.tile([1, ncalls], mybir.dt.int32)
            nc.sync.dma_start(out=gcnt_t[:], in_=gcnt_d[:])

            gq = [0]
            cnt_reg = nc.gpsimd.alloc_register("gcnt_reg")

            def issue_gathers(gi, gb):
                # single_packet packets hold <=64 descriptors per engine
                # (cap 7 tiles there); multi-packet is capped by the SWDGE
                # ring (1024 descriptors -> <64 tiles)
                cap = cfg.get("gather_tiles", 8)
                nq = cfg.get("n_swdge_queues", 1)
                ic0 = idx_off[gi][0]
                gcols = sum(tiles_gc[gi]) * (P // 16)
                # first call's columns as a tiny separate DMA so the gather
                # stream starts without waiting for the whole group's table
                c1 = min(cap * (P // 16), gcols)
                nc.scalar.dma_start(
                    out=idx_t[:, ic0 : ic0 + c1],
                    in_=gidx_d[:, ic0 : ic0 + c1],
                )
                if c1 < gcols:
                    nc.scalar.dma_start(
                        out=idx_t[:, ic0 + c1 : ic0 + gcols],
                        in_=gidx_d[:, ic0 + c1 : ic0 + gcols],
                    )
                for chi in range(nch):
                    nt = tiles_gc[gi][chi]
                    if nt == 0:
                        continue
                    to = seg_off[gi][chi]
                    ic = idx_off[gi][chi]
                    c0 = chi * CH
                    rows = min(CH, n_nodes - c0)
                    for j in range(0, nt, cap):
                        nj = min(cap, nt - j)
                        k = gq[0]
                        if not full_call[k]:
                            nc.gpsimd.reg_load(cnt_reg, gcnt_t[0:1, k : k + 1])
                        nc.gpsimd.dma_gather(
                            gb[
                                :, (to + j) * P : (to + j + nj) * P
                            ].rearrange("p (t e) -> p t e", e=P),
                            xbf_d[c0 : c0 + rows, :],
                            idx_t[
                                :,
                                ic + j * (P // 16) : ic + (j + nj) * (P // 16),
                            ],
                            nj * P,
                            (nj * P) if full_call[k] else cnt_reg,
                            P,
                            single_packet=cfg.get("gather_sp", True),
                            queue_num=k % nq,
                        )
                        gq[0] += 1

            def issue_smat(gi, sb):
                no = ops_per_g[gi]
                o0 = obase_g[gi]
                nc.sync.dma_start(
                    out=sb[:, : no * P],
                    in_=smat_d[:, o0 * P : (o0 + no) * P],
                )

            def alloc_gb(gi):
                gb = gpool.tile([P, tmax * P], BF16, tag="gb", name=f"gb{gi}")
                if gi < cfg.get("gbufs", 3):
                    # groups 0/1 gather idx-0 rows into their pad slots, but
                    # later (larger) groups in the same rotating buffer may
                    # reach beyond this group's tile range: clear that tail
                    tg = toff_g[gi + 1] - toff_g[gi]
                    if tg < tmax:
                        nc.vector.memset(gb[:, tg * P :], 0.0)
                return gb

            gbufs = {}
            gbufs[0] = alloc_gb(0)
            issue_gathers(0, gbufs[0])
            sbufs = {}
            sbufs[0] = spool.tile([P, omax * P], BF16, tag="sd", name="sd0")
            issue_smat(0, sbufs[0])

            if stage <= 1:
                # dump gathered tiles and skip compute entirely
                for gi in range(ng):
                    gb = gbufs.pop(gi)
                    if gi + 1 < ng:
                        gbufs[gi + 1] = gpool.tile(
                            [P, tmax * P], BF16, tag="gb", name=f"gb{gi + 1}"
                        )
                        issue_gathers(gi + 1, gbufs[gi + 1])
                    nt_g = toff_g[gi + 1] - toff_g[gi]
                    nc.sync.dma_start(
                        out=gdump_d[:, toff_g[gi] * P : toff_g[gi + 1] * P],
                        in_=gb[:, : nt_g * P],
                    )
            ng_eff = 0 if stage <= 1 else ng

            for gi in range(ng_eff):
                gb = gbufs.pop(gi)
                sd = sbufs.pop(gi)
                if gi + 1 < ng:
                    gbufs[gi + 1] = gpool.tile(
                        [P, tmax * P], BF16, tag="gb", name=f"gb{gi + 1}"
                    )
                    issue_gathers(gi + 1, gbufs[gi + 1])
                    sbufs[gi + 1] = spool.tile(
                        [P, omax * P], BF16, tag="sd", name=f"sd{gi + 1}"
                    )
                    issue_smat(gi + 1, sbufs[gi + 1])

                gwb = min(gw, nw - gi * gw)
                aggT = apool.tile([P, gw * P], F32, tag="aggT")
                for oi, (t, w, st, sp) in enumerate(ops_by_group[gi]):
                    tl = t - toff_g[gi]
                    wl = w - gi * gw
                    nc.tensor.matmul(
                        out=aggT[:, wl * P : (wl + 1) * P],
                        lhsT=gb[:, tl * P : (tl + 1) * P],
                        rhs=sd[:, oi * P : (oi + 1) * P],
                        start=st,
                        stop=sp,
                    )

                ag_sb = agpool.tile([P, gw * P], BF16, tag="agsb")
                evac_eng = nc.vector if cfg.get("evac_dve") else nc.scalar
                for wl in range(gwb):
                    if cfg.get("evac_dve"):
                        nc.vector.tensor_copy(
                            out=ag_sb[:, wl * P : (wl + 1) * P],
                            in_=aggT[:, wl * P : (wl + 1) * P],
                        )
                    else:
                        nc.scalar.activation(
                            out=ag_sb[:, wl * P : (wl + 1) * P],
                            in_=aggT[:, wl * P : (wl + 1) * P],
                            func=copyf,
                        )
                proj = ppool.tile([out_f, gw * P], F32, tag="proj")
                pn = cfg.get("proj_n", 4)  # windows per projection matmul
                for h in range(math.ceil(gw / pn)):
                    c0 = h * pn * P
                    c1 = min(gwb * P, (h + 1) * pn * P)
                    if c1 <= c0:
                        break
                    nc.tensor.matmul(
                        out=proj[:, c0:c1],
                        lhsT=wt_t[:],
                        rhs=ag_sb[:, c0:c1],
                        start=True,
                        stop=True,
                    )
                osb = opool.tile([out_f, gw * P], F32, tag="osb")
                if cfg.get("bias_dve"):
                    nc.vector.tensor_scalar(
                        out=osb[:, : gwb * P],
                        in0=proj[:, : gwb * P],
                        scalar1=bias_t[:],
                        scalar2=None,
                        op0=mybir.AluOpType.add,
                    )
                else:
                    nc.scalar.activation(
                        out=osb[:, : gwb * P],
                        in_=proj[:, : gwb * P],
                        func=mybir.ActivationFunctionType.Identity,
                        bias=bias_t[:],
                    )
                nc.sync.dma_start(
                    out=out_d[:, gi * gw * P : gi * gw * P + gwb * P],
                    in_=osb[:, : gwb * P],
                )
    return nc


def assemble_output(results, cfg):
    out_f = cfg["out_f"]
    ns, nw, ng, nch = _derived(cfg)
    blocks = []
    for r in results:
        o = np.asarray(r["out"], dtype=np.float32)  # [out_f, nw*P]
        blocks.append(o.T[:ns])
    return np.ascontiguousarray(np.concatenate(blocks, axis=0))


LAST_RESULTS = None


def kernel(x, weights, bias, adj_rows, adj_cols, adj_vals):
    global LAST_RESULTS
    cfg = default_cfg()
    in_maps, meta = prep_inputs(x, weights, bias, adj_rows, adj_cols, adj_vals, cfg)
    nc = bacc.Bacc(
        "TRN2",
        target_bir_lowering=False,
        debug=False,
        num_swdge_queues=cfg.get("n_swdge_queues", 1),
        dynamic_dma_scratch_size=32768,
    )
    build(nc, meta, cfg)
    nc.compile()
    res = None
    for attempt in range(3):
        try:
            res = bass_utils.run_bass_kernel_spmd(
                nc, in_maps, core_ids=list(range(cfg["n_cores"]))
            )
            break
        except Exception:
            # an earlier run can leave the exec unit wedged; a retry
            # (which triggers a device reset) normally recovers
            if attempt == 2:
                raise
    LAST_RESULTS = res
    return assemble_output(res.results, cfg)
